# revision 23
# baseline (speedup 1.0000x reference)
"""Trainium2 Bass kernel for DynamicFilterWithImageInput.

Model (per batch b):
  img_feat = mean_hw(relu(BN1(conv2d(raw_img, w_conv1, 3x3, zeropad=1) + b1)))   # (64,)
  df       = softmax_over_C(BN2(img_feat @ w_filt.T + b_filt).reshape(C, K*K))   # (C, 25)
  out      = depthwise_conv5x5(reflect_pad(x_feat), df)                          # (C, H, W)

Sharding: pure data-parallel over batch (16 batches -> 8 cores x 2 batches).

Depthwise engine split (per quad of QR=16 output rows):
  - PE: N_PE taps as diag-weight fp16 matmuls accumulating in PSUM
    (all odd-j taps go here so the chain engines keep 4B alignment).
  - ACT: evacuates the quad PSUM once into an fp16 seed tile E.
  - DVE: rows [0, RD): a seeded scalar_tensor_tensor chain, one tap per
    pass at the 2x (fp16/SBUF/step-1) perf mode, last pass writes ot_d.
  - GpSimd: rows [RD, 16): same seeded chain on its row slice -> ot_g.
  No cross-engine merge passes; ot_d/ot_g DMA straight to HBM.
"""

import sys

sys.path.insert(0, "/opt/trn_rl_repo")

import numpy as np

import concourse.bass as bass
import concourse.bacc as bacc
import concourse.mybir as mybir
import concourse.tile as tile
from concourse.bass_utils import run_bass_kernel_spmd
import concourse.dve_ops as _dve_ops


def _get_pair_mac():
    """Fused custom DVE op: out = in0*s0 + in1*s1 (two conv taps per pass).
    Registered dynamically so kernel.py is self-contained."""
    if hasattr(_dve_ops, "PAIR_MAC_ANT"):
        return _dve_ops.PAIR_MAC_ANT
    from concourse.dve_spec import Spec, Src0, C0, C1
    from concourse.dve_spec import Src1
    op = _dve_ops.DveOp(
        "PAIR_MAC_ANT",
        Spec(
            body=Src0 * C0 + Src1 * C1,
            reference=lambda in0, in1, s0, s1, imm2: (
                in0.astype(np.float32) * s0 + in1.astype(np.float32) * s1
            ).astype(np.float32),
        ),
        subdim=False,
        uops_sha={"v3": "f2ac165a27dbafb3", "v4": "49eb47656a95aba3"},
    )
    _dve_ops.OPS.append(op)
    _dve_ops.CUSTOM_DVE_SPECS[op.name] = op.spec
    _dve_ops._SUB_OPCODE_FOR_NAME[op.name] = (
        _dve_ops._CUSTOM_DVE_ROW_BASE + len(_dve_ops.OPS) - 1
    )
    assert max(_dve_ops._SUB_OPCODE_FOR_NAME.values()) < 0x20
    _dve_ops.PAIR_MAC_ANT = op
    return op


PAIR_MAC_ANT = _get_pair_mac()

F16 = mybir.dt.float16
F32 = mybir.dt.float32
AF = mybir.ActivationFunctionType
ALU = mybir.AluOpType

EPS = 1e-5
B_PC = 2          # batches per core
C = 256           # channels
CG = C // 128     # channel groups of 128
K5 = 5            # depthwise kernel size
NSLAB = B_PC * CG

_PROG_CACHE = {}


def _tap_split(n_pe, n_act, n_pair):
    """PE gets all odd-j taps plus extras (diag matmuls don't care about
    alignment); ACT gets n_act product taps; DVE gets n_pair PAIR_MAC pairs
    and the rest as scalar_tensor_tensor passes."""
    odd = [(i, j) for i in range(K5) for j in range(K5) if j % 2 == 1]
    even = [(i, j) for i in range(K5) for j in range(K5) if j % 2 == 0]
    assert n_pe >= len(odd), (n_pe, len(odd))
    pe_taps = odd + even[:n_pe - len(odd)]
    rest = even[n_pe - len(odd):]
    act_taps = rest[:n_act]
    pair_taps = rest[n_act:n_act + 2 * n_pair]
    stt_taps = rest[n_act + 2 * n_pair:]
    assert len(pair_taps) == 2 * n_pair
    return pe_taps, act_taps, pair_taps, stt_taps


def _build_program(H, W, n_pe=16, n_act=5, n_pair=2):
    Hp, Wp = H + 4, W + 4
    HWOUT = H * W
    GR = 4                               # rows per matmul (1 psum bank)
    QR = 16                              # output rows per quad (4 banks)
    QG = QR // GR
    assert H % QR == 0
    NQ = H // QR
    N1CH = min(512, HWOUT)               # conv1 psum chunk
    assert HWOUT % N1CH == 0
    N1 = HWOUT // N1CH                   # number of conv1 chunks
    IMCH = min(4 * N1CH, HWOUT)          # im2col streaming chunk

    pe_taps, act_taps, pair_taps, stt_taps = _tap_split(n_pe, n_act, n_pair)
    # PE-heavy split for the global last quad to shrink the DVE/DMA tail
    lq_split = _tap_split(min(n_pe + 4, 25 - max(n_act - 2, 0) - 2 * max(n_pair - 1, 0)),
                          max(n_act - 2, 0), max(n_pair - 1, 0))

    nc = bacc.Bacc("TRN2", target_bir_lowering=False, debug=False)

    x_d = nc.dram_tensor("x", [B_PC, C, Hp, Wp], F16, kind="ExternalInput").ap()
    df_d = nc.dram_tensor("df_sc", [B_PC, 25, C], F32).ap()
    im2col_d = nc.dram_tensor("im2col", [54, HWOUT], F16, kind="ExternalInput").ap()
    wconv_d = nc.dram_tensor("wconv", [54, 128], F16, kind="ExternalInput").ap()
    b1r_d = nc.dram_tensor("b1r", [128, 1], F32, kind="ExternalInput").ap()
    wffB_d = nc.dram_tensor("wffB", [128, 25, C], F16, kind="ExternalInput").ap()
    bias2_d = nc.dram_tensor("bias2", [64, C], F32, kind="ExternalInput").ap()
    bmask_d = nc.dram_tensor("bmask", [128, 2], F16, kind="ExternalInput").ap()
    ident_d = nc.dram_tensor("ident", [128, 128], F16, kind="ExternalInput").ap()
    id32_d = nc.dram_tensor("id32", [128, 128], F32, kind="ExternalInput").ap()
    out_d = nc.dram_tensor("out", [B_PC, C, H, W], F16, kind="ExternalOutput").ap()


    with tile.TileContext(nc) as tc:
        with (
            tc.tile_pool(name="consts", bufs=1) as consts,
            tc.tile_pool(name="p0", bufs=1) as p0,
            tc.tile_pool(name="imc", bufs=4) as imcp,
            tc.tile_pool(name="trash", bufs=2) as trashp,
            tc.tile_pool(name="xp", bufs=2) as xpp,
            tc.tile_pool(name="diag", bufs=2 * n_pe + 6) as diagp,
            tc.tile_pool(name="ev", bufs=2) as evp,
            tc.tile_pool(name="rch", bufs=2) as rchp,
            tc.tile_pool(name="ot", bufs=2) as otp,
            tc.tile_pool(name="proda", bufs=n_act + 1) as prodap,
            tc.tile_pool(name="prodq", bufs=n_pair + 1) as prodqp,
            tc.tile_pool(name="psum", bufs=2, space="PSUM") as psump,
        ):
            # ---------- phase-0 constants ----------
            wconv = consts.tile([54, 128], F16)
            b1r = consts.tile([128, 1], F32)
            wffB = consts.tile([128, 25, C], F16)
            bias2 = consts.tile([64, C], F32)
            bmask = consts.tile([128, 2], F16)
            ident = consts.tile([128, 128], F16)
            id32 = consts.tile([128, 128], F32)
            zeros = consts.tile([128, N1CH], F32)
            nc.sync.dma_start(wconv[:], wconv_d[:])
            nc.sync.dma_start(b1r[:], b1r_d[:])
            nc.gpsimd.memset(zeros[:], 0.0)

            # first im2col chunks go out before the big loads so conv1
            # starts immediately
            imts = {}
            for c4 in range(4):
                imt = imcp.tile([54, IMCH], F16, tag="imc", name="imt")
                nc.sync.dma_start(imt[:], im2col_d[:, c4 * IMCH:(c4 + 1) * IMCH])
                imts[c4] = imt
            nc.sync.dma_start(wffB[:], wffB_d[:])
            nc.sync.dma_start(bias2[:], bias2_d[:])
            nc.sync.dma_start(bmask[:], bmask_d[:])
            nc.sync.dma_start(ident[:], ident_d[:])
            nc.sync.dma_start(id32[:], id32_d[:])

            # ---------- depthwise input loads (start early; independent) ----
            xps = [None] * NSLAB

            def load_slab(s):
                b, cg = divmod(s, CG)
                xpf = xpp.tile([128, Hp * Wp + 8], F16, tag="xp", name="xpf")
                nc.vector.memset(xpf[:, Hp * Wp:], 0.0)
                xp3 = xpf[:, 0:Hp * Wp].rearrange("p (a b) -> p a b", a=Hp, b=Wp)
                nc.sync.dma_start(xp3, x_d[b, cg * 128:(cg + 1) * 128, :, :])
                xps[s] = (xpf, xp3)

            load_slab(0)
            # (slab 1 is loaded right after the conv1 stream is emitted)

            # ---------- conv1: streamed im2col, relu+sum split ACT/DVE ------
            acc = p0.tile([128, N1], F32)
            imt = None
            assert N1 % 4 == 0 and IMCH // N1CH == 4
            for c4 in range(N1 // 4):
                if c4 in imts:
                    imt = imts[c4]
                else:
                    imt = imcp.tile([54, IMCH], F16, tag="imc", name="imt")
                    nc.sync.dma_start(
                        imt[:], im2col_d[:, c4 * IMCH:(c4 + 1) * IMCH])
                ps1 = psump.tile([128, 4, N1CH], F32, tag="ps")
                for k in range(4):
                    nc.tensor.matmul(
                        ps1[:, k, :], wconv[:],
                        imt[:, k * N1CH:(k + 1) * N1CH],
                        start=True, stop=True,
                    )
                for k in range(4):
                    ci = c4 * 4 + k
                    tr = trashp.tile([128, N1CH], F32, tag="tr")
                    if ci % 2 == 0:
                        nc.scalar.activation(
                            tr[:], ps1[:, k, :], AF.Relu, bias=b1r[:], scale=1.0,
                            accum_out=acc[:, ci:ci + 1],
                        )
                    else:
                        nc.vector.scalar_tensor_tensor(
                            tr[:], ps1[:, k, :], b1r[:], zeros[:], ALU.add, ALU.max,
                            accum_out=acc[:, ci:ci + 1],
                        )
            load_slab(1)

            sfeat = p0.tile([128, 1], F32)
            if N1 > 1:
                nc.vector.tensor_reduce(sfeat[:], acc[:], mybir.AxisListType.X, ALU.add)
            else:
                nc.vector.tensor_copy(sfeat[:], acc[:])
            # masked lhsT [128,(b,oc) x 2]: column b = sums of batch b x 1/HW
            lhsT2 = p0.tile([128, B_PC], F16)
            nc.vector.tensor_scalar(lhsT2[:], bmask[:], sfeat[:], None, ALU.mult)

            # softmax input [b*32 + t, c] is scattered straight from the
            # dense evacuation tiles (SBUF->SBUF DMA)
            dfsb = p0.tile([B_PC * 32, C], F32)

            # dense: df[b, t, c]; 4 x 512 chunks per psum slot
            CH_PER_SLOT = 4
            t0 = 0
            while t0 < 25:
                tws = []
                t1 = t0
                while t1 < 25 and len(tws) < CH_PER_SLOT:
                    tw = min(2, 25 - t1)
                    tws.append((t1, tw))
                    t1 += tw
                tot = t1 - t0
                psd = psump.tile([B_PC, tot, C], F32, tag="ps")
                for (tt, tw) in tws:
                    nc.tensor.matmul(
                        psd[:, tt - t0:tt - t0 + tw, :], lhsT2[:],
                        wffB[:, tt:tt + tw, :],
                        start=True, stop=True,
                    )
                dfc = trashp.tile([B_PC, tot, C], F32, tag="dfc")
                nc.scalar.copy(dfc[:], psd[:])
                nc.sync.dma_start(df_d[:, t0:t0 + tot, :], dfc[:])
                t0 = t1

            # softmax over channels; batch b parked at partition b*32 so the
            # PE transpose below sees base partitions in {0, 32}
            edf = p0.tile([B_PC * 32, C], F32)
            ssum = p0.tile([B_PC * 32, 1], F32)
            rsum = p0.tile([B_PC * 32, 1], F32)
            wsm = p0.tile([B_PC * 32, C], F32)
            for b in range(B_PC):
                sl = slice(b * 32, b * 32 + 25)
                nc.sync.dma_start(dfsb[sl, :], df_d[b])
            nc.vector.tensor_tensor(dfsb[0:64, :], dfsb[0:64, :], bias2[:],
                                    ALU.add)
            for b in range(B_PC):
                sl = slice(b * 32, b * 32 + 25)
                nc.scalar.activation(edf[sl, :], dfsb[sl, :], AF.Exp)
                nc.vector.tensor_reduce(
                    ssum[sl, :], edf[sl, :], mybir.AxisListType.X, ALU.add)
                nc.vector.reciprocal(rsum[sl, :], ssum[sl, :])
                nc.vector.tensor_scalar(
                    wsm[sl, :], edf[sl, :], rsum[sl, :], None, ALU.mult)

            # per-slab filter values [128(c), 25] via PE transpose (no bounce)
            vts = []
            for s in range(NSLAB):
                b, cg = divmod(s, CG)
                pst = psump.tile([128, 25], F32, tag="ps")
                nc.tensor.transpose(
                    pst[:], wsm[b * 32:b * 32 + 25, cg * 128:(cg + 1) * 128],
                    id32[b * 32:b * 32 + 25, 0:25],
                )
                vt = p0.tile([128, 25], F32, tag=f"vt{s}")
                nc.scalar.copy(vt[:], pst[:])
                vts.append(vt)

            # diag tiles for the PE taps only (last slab also covers the
            # PE-heavy final-quad split)
            dts_all = []
            for s in range(NSLAB):
                taps = set(pe_taps)
                if s == NSLAB - 1:
                    taps |= set(lq_split[0])
                dts = {}
                for (i, j) in sorted(taps):
                    t = i * K5 + j
                    dt_ = diagp.tile([128, 128], F16, tag="dt")
                    nc.scalar.mul(dt_[:], ident[:], vts[s][:, t:t + 1])
                    dts[t] = dt_
                dts_all.append(dts)

            # ---------- depthwise ----------
            for s in range(NSLAB):
                b, cg = divmod(s, CG)
                vt = vts[s]
                xpf, xp = xps[s]
                dts = dts_all[s]
                if 1 <= s < NSLAB - 1:
                    load_slab(s + 1)

                for q in range(NQ):
                    if s == NSLAB - 1 and q == NQ - 1:
                        q_pe, q_act, q_pair, q_stt = lq_split
                    else:
                        q_pe, q_act, q_pair, q_stt = (
                            pe_taps, act_taps, pair_taps, stt_taps)
                    y0 = q * QR

                    def win(i, j):
                        return xp[:, y0 + i:y0 + i + QR, j:j + W]

                    def sc(i, j):
                        t = i * K5 + j
                        return vt[:, t:t + 1]

                    # PE taps: diag matmuls accumulate into the 4 psum banks
                    ps = psump.tile([128, QR, W], F32, tag="ps")
                    for g in range(QG):
                        gy = y0 + g * GR
                        for k, (i, j) in enumerate(q_pe):
                            nc.tensor.matmul(
                                ps[:, g * GR:(g + 1) * GR, :],
                                dts[i * K5 + j][:],
                                xp[:, gy + i:gy + i + GR, j:j + W],
                                start=(k == 0),
                                stop=(k == len(q_pe) - 1),
                            )

                    # partial-product tiles (clean [128, QR, W] layouts):
                    # ACT products + DVE pair products
                    parts = []
                    for (i, j) in q_act:
                        pa = prodap.tile([128, QR, W], F16, tag="pa", name="pa")
                        nc.scalar.activation(
                            pa[:], win(i, j), AF.Copy, scale=sc(i, j))
                        parts.append(pa[:])
                    # pairs read flat full-padded-width runs (junk columns
                    # are sliced away at absorb time)
                    def run(i, j):
                        off = (y0 + i) * Wp + j
                        return xpf[:, off:off + QR * Wp]

                    for pi in range(len(q_pair) // 2):
                        ia, ja = q_pair[2 * pi]
                        ib, jb = q_pair[2 * pi + 1]
                        pq = prodqp.tile([128, QR * Wp], F16, tag="pq", name="pq")
                        nc.vector._custom_dve(
                            PAIR_MAC_ANT, out=pq[:],
                            in0=run(ia, ja), in1=run(ib, jb),
                            s0=sc(ia, ja), s1=sc(ib, jb))
                        parts.append(pq[:].rearrange(
                            "p (a b) -> p a b", a=QR, b=Wp)[:, :, 0:W])

                    # ACT: evacuate quad PSUM once -> fp16 seed tile E
                    ev = evp.tile([128, QR, W], F16, tag="ev")
                    nc.scalar.copy(ev[:], ps[:])

                    # DVE chain: absorb partials with 2x tensor_tensor adds,
                    # then any leftover taps as scalar_tensor_tensor passes
                    ot = otp.tile([128, QR, W], F16, tag="ot")
                    n_steps = len(parts) + len(q_stt)
                    assert n_steps > 0
                    prev = ev[:]
                    step = 0
                    for pa in parts:
                        step += 1
                        dst = ot[:] if step == n_steps else rchp.tile(
                            [128, QR, W], F16, tag="rch", name="rch")[:]
                        nc.vector.tensor_tensor(dst, prev, pa, ALU.add)
                        prev = dst
                    for (i, j) in q_stt:
                        step += 1
                        dst = ot[:] if step == n_steps else rchp.tile(
                            [128, QR, W], F16, tag="rch", name="rch")[:]
                        nc.vector.scalar_tensor_tensor(
                            dst, win(i, j), sc(i, j), prev, ALU.mult, ALU.add)
                        prev = dst
                    nc.sync.dma_start(
                        out_d[b, cg * 128:(cg + 1) * 128, y0:y0 + QR, :],
                        ot[:])

    nc.compile()
    return nc


def get_program(H, W, n_pe=16, n_act=5, n_pair=2):
    key = (H, W, n_pe, n_act, n_pair)
    if key not in _PROG_CACHE:
        _PROG_CACHE[key] = _build_program(H, W, n_pe, n_act, n_pair)
    return _PROG_CACHE[key]


def host_prep(x_feat, raw_img, w_conv1, b_conv1, g1, beta1, m1, v1,
              w_filt, b_filt, g2, beta2, m2, v2):
    """Fold BN params, build im2col + packed weights; returns per-core in_maps."""
    B, Cc, H, W = x_feat.shape
    assert Cc == C
    n_cores = B // B_PC

    a1 = g1 / np.sqrt(v1 + EPS)
    w1f = (w_conv1 * a1[:, None, None, None]).astype(np.float32)   # (64,3,3,3)
    b1f = (b_conv1 - m1) * a1 + beta1                               # (64,)

    a2 = g2 / np.sqrt(v2 + EPS)
    wff = (w_filt * a2[:, None]).astype(np.float32)                 # (6400,64)
    bff = (b_filt - m2) * a2 + beta2                                # (6400,)

    # wffB[(b,oc), t, c] = wff[c*25+t, oc] replicated for both batches;
    # BN2 bias folded separately via bias2 (added before the softmax exp)
    w64 = wff.reshape(C, 25, 64).transpose(2, 1, 0)          # (64, 25, C)
    wffB = np.concatenate([w64, w64], axis=0).astype(np.float16)
    bias2 = np.zeros((64, C), np.float32)
    bias2[0:25] = bff.reshape(C, 25).T
    bias2[32:57] = bff.reshape(C, 25).T
    bmask = np.zeros((128, B_PC), np.float16)
    for b in range(B_PC):
        bmask[b * 64:(b + 1) * 64, b] = 1.0 / (x_feat.shape[2] * x_feat.shape[3])

    b1r = np.tile(b1f, B_PC).reshape(128, 1).astype(np.float32)

    ident = np.eye(128, dtype=np.float16)
    # 25x25 identity blocks at partition offsets 0 and 32 (PE-transpose
    # requires the identity operand at the same base partition as the input)
    id32 = np.zeros((128, 128), np.float32)
    for b in range(B_PC):
        id32[b * 32:b * 32 + 25, 0:25] = np.eye(25)

    xpad16 = np.pad(x_feat, ((0, 0), (0, 0), (2, 2), (2, 2)),
                    mode="reflect").astype(np.float16)

    # conv1 im2col, zero pad 1: [54, H*W] per core
    rawpad = np.pad(raw_img, ((0, 0), (0, 0), (1, 1), (1, 1))).astype(np.float32)

    # wconv[b*27 + (c*9+i*3+j), b*64+o] = w1f[o, c, i, j]
    wconv = np.zeros((54, 128), np.float32)
    w_flat = w1f.transpose(1, 2, 3, 0).reshape(27, 64)  # (c*9+i*3+j, o)
    for b in range(B_PC):
        wconv[b * 27:(b + 1) * 27, b * 64:(b + 1) * 64] = w_flat
    wconv16 = wconv.astype(np.float16)

    in_maps = []
    for core in range(n_cores):
        bs = core * B_PC
        im2col = np.empty((54, H * W), np.float32)
        for b in range(B_PC):
            for c in range(3):
                for i in range(3):
                    for j in range(3):
                        p = b * 27 + c * 9 + i * 3 + j
                        im2col[p] = rawpad[bs + b, c, i:i + H, j:j + W].reshape(-1)
        in_maps.append({
            "x": xpad16[bs:bs + B_PC],
            "im2col": im2col.astype(np.float16),
            "wconv": wconv16,
            "b1r": b1r,
            "wffB": wffB,
            "bias2": bias2,
            "bmask": bmask,
            "ident": ident,
            "id32": id32,
        })
    return in_maps


def run(inputs, trace=False, n_pe=16, n_act=5, n_pair=2):
    x_feat = inputs["x_feat"]
    B, _, H, W = x_feat.shape
    nc = get_program(H, W, n_pe, n_act, n_pair)
    in_maps = host_prep(**inputs)
    n_cores = len(in_maps)
    res = run_bass_kernel_spmd(nc, in_maps, list(range(n_cores)), trace=trace)
    out = np.concatenate(
        [r["out"].astype(np.float32) for r in res.results], axis=0)
    return out, res


def kernel(**inputs) -> np.ndarray:
    out, _ = run(inputs, trace=False)
    return out


# revision 24
# speedup vs baseline: 1.1896x; 1.1896x over previous
"""Trainium2 Bass kernel for DynamicFilterWithImageInput.

Model (per batch b):
  img_feat = mean_hw(relu(BN1(conv2d(raw_img, w_conv1, 3x3, zeropad=1) + b1)))   # (64,)
  df       = softmax_over_C(BN2(img_feat @ w_filt.T + b_filt).reshape(C, K*K))   # (C, 25)
  out      = depthwise_conv5x5(reflect_pad(x_feat), df)                          # (C, H, W)

Sharding: pure data-parallel over batch (16 batches -> 8 cores x 2 batches).

Depthwise engine split (per quad of QR=16 output rows):
  - PE: N_PE taps as diag-weight fp16 matmuls accumulating in PSUM
    (all odd-j taps go here so the chain engines keep 4B alignment).
  - ACT: evacuates the quad PSUM once into an fp16 seed tile E.
  - DVE: rows [0, RD): a seeded scalar_tensor_tensor chain, one tap per
    pass at the 2x (fp16/SBUF/step-1) perf mode, last pass writes ot_d.
  - GpSimd: rows [RD, 16): same seeded chain on its row slice -> ot_g.
  No cross-engine merge passes; ot_d/ot_g DMA straight to HBM.
"""

import sys

sys.path.insert(0, "/opt/trn_rl_repo")

import numpy as np

import concourse.bass as bass
import concourse.bacc as bacc
import concourse.mybir as mybir
import concourse.tile as tile
from concourse.bass_utils import run_bass_kernel_spmd
import concourse.dve_ops as _dve_ops


def _get_pair_mac():
    """Fused custom DVE op: out = in0*s0 + in1*s1 (two conv taps per pass).
    Registered dynamically so kernel.py is self-contained."""
    if hasattr(_dve_ops, "PAIR_MAC_ANT"):
        return _dve_ops.PAIR_MAC_ANT
    from concourse.dve_spec import Spec, Src0, C0, C1
    from concourse.dve_spec import Src1
    op = _dve_ops.DveOp(
        "PAIR_MAC_ANT",
        Spec(
            body=Src0 * C0 + Src1 * C1,
            reference=lambda in0, in1, s0, s1, imm2: (
                in0.astype(np.float32) * s0 + in1.astype(np.float32) * s1
            ).astype(np.float32),
        ),
        subdim=False,
        uops_sha={"v3": "f2ac165a27dbafb3", "v4": "49eb47656a95aba3"},
    )
    _dve_ops.OPS.append(op)
    _dve_ops.CUSTOM_DVE_SPECS[op.name] = op.spec
    _dve_ops._SUB_OPCODE_FOR_NAME[op.name] = (
        _dve_ops._CUSTOM_DVE_ROW_BASE + len(_dve_ops.OPS) - 1
    )
    assert max(_dve_ops._SUB_OPCODE_FOR_NAME.values()) < 0x20
    _dve_ops.PAIR_MAC_ANT = op
    return op


PAIR_MAC_ANT = _get_pair_mac()

F16 = mybir.dt.float16
F32 = mybir.dt.float32
AF = mybir.ActivationFunctionType
ALU = mybir.AluOpType

EPS = 1e-5
B_PC = 2          # batches per core
C = 256           # channels
CG = C // 128     # channel groups of 128
K5 = 5            # depthwise kernel size
NSLAB = B_PC * CG

_PROG_CACHE = {}


def _tap_split(n_pe, n_act, n_pair):
    """PE gets all odd-j taps plus extras (diag matmuls don't care about
    alignment); ACT gets n_act product taps; DVE gets n_pair PAIR_MAC pairs
    and the rest as scalar_tensor_tensor passes."""
    odd = [(i, j) for i in range(K5) for j in range(K5) if j % 2 == 1]
    even = [(i, j) for i in range(K5) for j in range(K5) if j % 2 == 0]
    assert n_pe >= len(odd), (n_pe, len(odd))
    pe_taps = odd + even[:n_pe - len(odd)]
    rest = even[n_pe - len(odd):]
    act_taps = rest[:n_act]
    pair_taps = rest[n_act:n_act + 2 * n_pair]
    stt_taps = rest[n_act + 2 * n_pair:]
    assert len(pair_taps) == 2 * n_pair
    return pe_taps, act_taps, pair_taps, stt_taps


def _build_program(H, W, n_pe=16, n_act=5, n_pair=2):
    Hp, Wp = H + 4, W + 4
    HWOUT = H * W
    GR = 4                               # rows per matmul (1 psum bank)
    QR = 16                              # output rows per quad (4 banks)
    QG = QR // GR
    assert H % QR == 0
    NQ = H // QR
    N1CH = min(512, HWOUT)               # conv1 psum chunk
    assert HWOUT % N1CH == 0
    N1 = HWOUT // N1CH                   # number of conv1 chunks
    IMCH = min(4 * N1CH, HWOUT)          # im2col streaming chunk

    pe_taps, act_taps, pair_taps, stt_taps = _tap_split(n_pe, n_act, n_pair)
    # PE-heavy split for the global last quad to shrink the DVE/DMA tail
    lq_split = _tap_split(min(n_pe + 4, 25 - max(n_act - 2, 0) - 2 * max(n_pair - 1, 0)),
                          max(n_act - 2, 0), max(n_pair - 1, 0))

    nc = bacc.Bacc("TRN2", target_bir_lowering=False, debug=False)

    x_d = nc.dram_tensor("x", [B_PC, C, Hp, Wp], F16, kind="ExternalInput").ap()
    df_d = nc.dram_tensor("df_sc", [B_PC, 25, C], F32).ap()
    im2col_d = nc.dram_tensor("im2col", [54, HWOUT], F16, kind="ExternalInput").ap()
    wconv_d = nc.dram_tensor("wconv", [54, 128], F16, kind="ExternalInput").ap()
    b1r_d = nc.dram_tensor("b1r", [128, 1], F32, kind="ExternalInput").ap()
    wffB_d = nc.dram_tensor("wffB", [128, 25, C], F16, kind="ExternalInput").ap()
    bias2_d = nc.dram_tensor("bias2", [64, C], F32, kind="ExternalInput").ap()
    bmask_d = nc.dram_tensor("bmask", [128, 2], F16, kind="ExternalInput").ap()
    ident_d = nc.dram_tensor("ident", [128, 128], F16, kind="ExternalInput").ap()
    id32_d = nc.dram_tensor("id32", [128, 128], F32, kind="ExternalInput").ap()
    out_d = nc.dram_tensor("out", [B_PC, C, H, W], F16, kind="ExternalOutput").ap()


    with tile.TileContext(nc) as tc:
        with (
            tc.tile_pool(name="consts", bufs=1) as consts,
            tc.tile_pool(name="p0", bufs=1) as p0,
            tc.tile_pool(name="imc", bufs=4) as imcp,
            tc.tile_pool(name="trash", bufs=2) as trashp,
            tc.tile_pool(name="xp", bufs=2) as xpp,
            tc.tile_pool(name="diag", bufs=2 * n_pe + 6) as diagp,
            tc.tile_pool(name="ev", bufs=2) as evp,
            tc.tile_pool(name="rch", bufs=2) as rchp,
            tc.tile_pool(name="ot", bufs=2) as otp,
            tc.tile_pool(name="proda", bufs=n_act + 1) as prodap,
            tc.tile_pool(name="prodq", bufs=n_pair + 1) as prodqp,
            tc.tile_pool(name="psum", bufs=2, space="PSUM") as psump,
        ):
            # ---------- phase-0 constants ----------
            wconv = consts.tile([54, 128], F16)
            b1r = consts.tile([128, 1], F32)
            wffB = consts.tile([128, 25, C], F16)
            bias2 = consts.tile([64, C], F32)
            bmask = consts.tile([128, 2], F16)
            ident = consts.tile([128, 128], F16)
            id32 = consts.tile([128, 128], F32)
            zeros = consts.tile([128, N1CH], F32)
            nc.sync.dma_start(wconv[:], wconv_d[:])
            nc.sync.dma_start(b1r[:], b1r_d[:])
            nc.gpsimd.memset(zeros[:], 0.0)

            # first im2col chunks go out before the big loads so conv1
            # starts immediately
            imts = {}
            for c4 in range(4):
                imt = imcp.tile([54, IMCH], F16, tag="imc", name="imt")
                nc.sync.dma_start(imt[:], im2col_d[:, c4 * IMCH:(c4 + 1) * IMCH])
                imts[c4] = imt
            nc.sync.dma_start(wffB[:], wffB_d[:])
            nc.sync.dma_start(bias2[:], bias2_d[:])
            nc.sync.dma_start(bmask[:], bmask_d[:])
            nc.sync.dma_start(ident[:], ident_d[:])
            nc.sync.dma_start(id32[:], id32_d[:])

            # ---------- depthwise input loads (start early; independent) ----
            xps = [None] * NSLAB

            def load_slab(s):
                b, cg = divmod(s, CG)
                xpf = xpp.tile([128, Hp * Wp + 8], F16, tag="xp", name="xpf")
                nc.vector.memset(xpf[:, Hp * Wp:], 0.0)
                xp3 = xpf[:, 0:Hp * Wp].rearrange("p (a b) -> p a b", a=Hp, b=Wp)
                nc.sync.dma_start(xp3, x_d[b, cg * 128:(cg + 1) * 128, :, :])
                xps[s] = (xpf, xp3)

            load_slab(0)
            # (slab 1 is loaded right after the conv1 stream is emitted)

            # ---------- conv1: streamed im2col, relu+sum split ACT/DVE ------
            # two accumulator tiles so the ACT and DVE absorbs don't
            # serialize on a shared output tile
            accA = p0.tile([128, N1 // 2], F32)
            accB = p0.tile([128, N1 // 2], F32)
            imt = None
            assert N1 % 4 == 0 and IMCH // N1CH == 4
            for c4 in range(N1 // 4):
                if c4 in imts:
                    imt = imts[c4]
                else:
                    imt = imcp.tile([54, IMCH], F16, tag="imc", name="imt")
                    nc.sync.dma_start(
                        imt[:], im2col_d[:, c4 * IMCH:(c4 + 1) * IMCH])
                ps1 = psump.tile([128, 4, N1CH], F32, tag="ps")
                for k in range(4):
                    nc.tensor.matmul(
                        ps1[:, k, :], wconv[:],
                        imt[:, k * N1CH:(k + 1) * N1CH],
                        start=True, stop=True,
                    )
                for k in range(4):
                    ci = c4 * 4 + k
                    tr = trashp.tile([128, N1CH], F32, tag="tr")
                    if ci % 2 == 0:
                        nc.scalar.activation(
                            tr[:], ps1[:, k, :], AF.Relu, bias=b1r[:], scale=1.0,
                            accum_out=accA[:, ci // 2:ci // 2 + 1],
                        )
                    else:
                        nc.vector.scalar_tensor_tensor(
                            tr[:], ps1[:, k, :], b1r[:], zeros[:], ALU.add, ALU.max,
                            accum_out=accB[:, ci // 2:ci // 2 + 1],
                        )
            load_slab(1)

            sfeat = p0.tile([128, 1], F32)
            sfB = p0.tile([128, 1], F32)
            nc.vector.tensor_reduce(sfeat[:], accA[:], mybir.AxisListType.X, ALU.add)
            nc.vector.tensor_reduce(sfB[:], accB[:], mybir.AxisListType.X, ALU.add)
            nc.vector.tensor_tensor(sfeat[:], sfeat[:], sfB[:], ALU.add)
            # masked lhsT [128,(b,oc) x 2]: column b = sums of batch b x 1/HW
            lhsT2 = p0.tile([128, B_PC], F16)
            nc.vector.tensor_scalar(lhsT2[:], bmask[:], sfeat[:], None, ALU.mult)

            # softmax input [b*32 + t, c] is scattered straight from the
            # dense evacuation tiles (SBUF->SBUF DMA)
            dfsb = p0.tile([B_PC * 32, C], F32)

            # dense: df[b, t, c]; 4 x 512 chunks per psum slot
            CH_PER_SLOT = 4
            t0 = 0
            while t0 < 25:
                tws = []
                t1 = t0
                while t1 < 25 and len(tws) < CH_PER_SLOT:
                    tw = min(2, 25 - t1)
                    tws.append((t1, tw))
                    t1 += tw
                tot = t1 - t0
                psd = psump.tile([B_PC, tot, C], F32, tag="ps")
                for (tt, tw) in tws:
                    nc.tensor.matmul(
                        psd[:, tt - t0:tt - t0 + tw, :], lhsT2[:],
                        wffB[:, tt:tt + tw, :],
                        start=True, stop=True,
                    )
                dfc = trashp.tile([B_PC, tot, C], F32, tag="dfc")
                nc.scalar.copy(dfc[:], psd[:])
                nc.sync.dma_start(df_d[:, t0:t0 + tot, :], dfc[:])
                t0 = t1

            # softmax over channels; batch b parked at partition b*32 so the
            # PE transpose below sees base partitions in {0, 32}
            edf = p0.tile([B_PC * 32, C], F32)
            ssum = p0.tile([B_PC * 32, 1], F32)
            rsum = p0.tile([B_PC * 32, 1], F32)
            wsm = p0.tile([B_PC * 32, C], F32)
            for b in range(B_PC):
                sl = slice(b * 32, b * 32 + 25)
                nc.sync.dma_start(dfsb[sl, :], df_d[b])
            nc.vector.tensor_tensor(dfsb[0:64, :], dfsb[0:64, :], bias2[:],
                                    ALU.add)
            for b in range(B_PC):
                sl = slice(b * 32, b * 32 + 25)
                nc.scalar.activation(edf[sl, :], dfsb[sl, :], AF.Exp)
                nc.vector.tensor_reduce(
                    ssum[sl, :], edf[sl, :], mybir.AxisListType.X, ALU.add)
                nc.vector.reciprocal(rsum[sl, :], ssum[sl, :])
                nc.vector.tensor_scalar(
                    wsm[sl, :], edf[sl, :], rsum[sl, :], None, ALU.mult)

            # per-slab filter values [128(c), 25] via PE transpose (no bounce)
            vts = []
            for s in range(NSLAB):
                b, cg = divmod(s, CG)
                pst = psump.tile([128, 25], F32, tag="ps")
                nc.tensor.transpose(
                    pst[:], wsm[b * 32:b * 32 + 25, cg * 128:(cg + 1) * 128],
                    id32[b * 32:b * 32 + 25, 0:25],
                )
                vt = p0.tile([128, 25], F32, tag=f"vt{s}")
                nc.scalar.copy(vt[:], pst[:])
                vts.append(vt)

            # diag tiles for the PE taps only (last slab also covers the
            # PE-heavy final-quad split)
            dts_all = []
            for s in range(NSLAB):
                taps = set(pe_taps)
                if s == NSLAB - 1:
                    taps |= set(lq_split[0])
                dts = {}
                for (i, j) in sorted(taps):
                    t = i * K5 + j
                    dt_ = diagp.tile([128, 128], F16, tag="dt")
                    nc.scalar.mul(dt_[:], ident[:], vts[s][:, t:t + 1])
                    dts[t] = dt_
                dts_all.append(dts)

            # ---------- depthwise ----------
            for s in range(NSLAB):
                b, cg = divmod(s, CG)
                vt = vts[s]
                xpf, xp = xps[s]
                dts = dts_all[s]
                if 1 <= s < NSLAB - 1:
                    load_slab(s + 1)

                for q in range(NQ):
                    if s == NSLAB - 1 and q == NQ - 1:
                        q_pe, q_act, q_pair, q_stt = lq_split
                    else:
                        q_pe, q_act, q_pair, q_stt = (
                            pe_taps, act_taps, pair_taps, stt_taps)
                    y0 = q * QR

                    def win(i, j):
                        return xp[:, y0 + i:y0 + i + QR, j:j + W]

                    def sc(i, j):
                        t = i * K5 + j
                        return vt[:, t:t + 1]

                    # PE taps: diag matmuls accumulate into the 4 psum banks
                    ps = psump.tile([128, QR, W], F32, tag="ps")
                    for g in range(QG):
                        gy = y0 + g * GR
                        for k, (i, j) in enumerate(q_pe):
                            nc.tensor.matmul(
                                ps[:, g * GR:(g + 1) * GR, :],
                                dts[i * K5 + j][:],
                                xp[:, gy + i:gy + i + GR, j:j + W],
                                start=(k == 0),
                                stop=(k == len(q_pe) - 1),
                            )

                    # partial-product tiles (clean [128, QR, W] layouts):
                    # ACT products + DVE pair products
                    parts = []
                    for (i, j) in q_act:
                        pa = prodap.tile([128, QR, W], F16, tag="pa", name="pa")
                        nc.scalar.activation(
                            pa[:], win(i, j), AF.Copy, scale=sc(i, j))
                        parts.append(pa[:])
                    # pairs read flat full-padded-width runs (junk columns
                    # are sliced away at absorb time)
                    def run(i, j):
                        off = (y0 + i) * Wp + j
                        return xpf[:, off:off + QR * Wp]

                    for pi in range(len(q_pair) // 2):
                        ia, ja = q_pair[2 * pi]
                        ib, jb = q_pair[2 * pi + 1]
                        pq = prodqp.tile([128, QR * Wp], F16, tag="pq", name="pq")
                        nc.vector._custom_dve(
                            PAIR_MAC_ANT, out=pq[:],
                            in0=run(ia, ja), in1=run(ib, jb),
                            s0=sc(ia, ja), s1=sc(ib, jb))
                        parts.append(pq[:].rearrange(
                            "p (a b) -> p a b", a=QR, b=Wp)[:, :, 0:W])

                    # ACT: evacuate quad PSUM once -> fp16 seed tile E
                    ev = evp.tile([128, QR, W], F16, tag="ev")
                    nc.scalar.copy(ev[:], ps[:])

                    # DVE chain: absorb partials with 2x tensor_tensor adds,
                    # then any leftover taps as scalar_tensor_tensor passes
                    ot = otp.tile([128, QR, W], F16, tag="ot")
                    n_steps = len(parts) + len(q_stt)
                    assert n_steps > 0
                    prev = ev[:]
                    step = 0
                    for pa in parts:
                        step += 1
                        dst = ot[:] if step == n_steps else rchp.tile(
                            [128, QR, W], F16, tag="rch", name="rch")[:]
                        nc.vector.tensor_tensor(dst, prev, pa, ALU.add)
                        prev = dst
                    for (i, j) in q_stt:
                        step += 1
                        dst = ot[:] if step == n_steps else rchp.tile(
                            [128, QR, W], F16, tag="rch", name="rch")[:]
                        nc.vector.scalar_tensor_tensor(
                            dst, win(i, j), sc(i, j), prev, ALU.mult, ALU.add)
                        prev = dst
                    nc.sync.dma_start(
                        out_d[b, cg * 128:(cg + 1) * 128, y0:y0 + QR, :],
                        ot[:])

    nc.compile()
    return nc


def get_program(H, W, n_pe=16, n_act=5, n_pair=2):
    key = (H, W, n_pe, n_act, n_pair)
    if key not in _PROG_CACHE:
        _PROG_CACHE[key] = _build_program(H, W, n_pe, n_act, n_pair)
    return _PROG_CACHE[key]


def host_prep(x_feat, raw_img, w_conv1, b_conv1, g1, beta1, m1, v1,
              w_filt, b_filt, g2, beta2, m2, v2):
    """Fold BN params, build im2col + packed weights; returns per-core in_maps."""
    B, Cc, H, W = x_feat.shape
    assert Cc == C
    n_cores = B // B_PC

    a1 = g1 / np.sqrt(v1 + EPS)
    w1f = (w_conv1 * a1[:, None, None, None]).astype(np.float32)   # (64,3,3,3)
    b1f = (b_conv1 - m1) * a1 + beta1                               # (64,)

    a2 = g2 / np.sqrt(v2 + EPS)
    wff = (w_filt * a2[:, None]).astype(np.float32)                 # (6400,64)
    bff = (b_filt - m2) * a2 + beta2                                # (6400,)

    # wffB[(b,oc), t, c] = wff[c*25+t, oc] replicated for both batches;
    # BN2 bias folded separately via bias2 (added before the softmax exp)
    w64 = wff.reshape(C, 25, 64).transpose(2, 1, 0)          # (64, 25, C)
    wffB = np.concatenate([w64, w64], axis=0).astype(np.float16)
    bias2 = np.zeros((64, C), np.float32)
    bias2[0:25] = bff.reshape(C, 25).T
    bias2[32:57] = bff.reshape(C, 25).T
    bmask = np.zeros((128, B_PC), np.float16)
    for b in range(B_PC):
        bmask[b * 64:(b + 1) * 64, b] = 1.0 / (x_feat.shape[2] * x_feat.shape[3])

    b1r = np.tile(b1f, B_PC).reshape(128, 1).astype(np.float32)

    ident = np.eye(128, dtype=np.float16)
    # 25x25 identity blocks at partition offsets 0 and 32 (PE-transpose
    # requires the identity operand at the same base partition as the input)
    id32 = np.zeros((128, 128), np.float32)
    for b in range(B_PC):
        id32[b * 32:b * 32 + 25, 0:25] = np.eye(25)

    xpad16 = np.pad(x_feat, ((0, 0), (0, 0), (2, 2), (2, 2)),
                    mode="reflect").astype(np.float16)

    # conv1 im2col, zero pad 1: [54, H*W] per core
    rawpad = np.pad(raw_img, ((0, 0), (0, 0), (1, 1), (1, 1))).astype(np.float32)

    # wconv[b*27 + (c*9+i*3+j), b*64+o] = w1f[o, c, i, j]
    wconv = np.zeros((54, 128), np.float32)
    w_flat = w1f.transpose(1, 2, 3, 0).reshape(27, 64)  # (c*9+i*3+j, o)
    for b in range(B_PC):
        wconv[b * 27:(b + 1) * 27, b * 64:(b + 1) * 64] = w_flat
    wconv16 = wconv.astype(np.float16)

    in_maps = []
    for core in range(n_cores):
        bs = core * B_PC
        im2col = np.empty((54, H * W), np.float32)
        for b in range(B_PC):
            for c in range(3):
                for i in range(3):
                    for j in range(3):
                        p = b * 27 + c * 9 + i * 3 + j
                        im2col[p] = rawpad[bs + b, c, i:i + H, j:j + W].reshape(-1)
        in_maps.append({
            "x": xpad16[bs:bs + B_PC],
            "im2col": im2col.astype(np.float16),
            "wconv": wconv16,
            "b1r": b1r,
            "wffB": wffB,
            "bias2": bias2,
            "bmask": bmask,
            "ident": ident,
            "id32": id32,
        })
    return in_maps


def run(inputs, trace=False, n_pe=16, n_act=5, n_pair=2):
    x_feat = inputs["x_feat"]
    B, _, H, W = x_feat.shape
    nc = get_program(H, W, n_pe, n_act, n_pair)
    in_maps = host_prep(**inputs)
    n_cores = len(in_maps)
    res = run_bass_kernel_spmd(nc, in_maps, list(range(n_cores)), trace=trace)
    out = np.concatenate(
        [r["out"].astype(np.float32) for r in res.results], axis=0)
    return out, res


def kernel(**inputs) -> np.ndarray:
    out, _ = run(inputs, trace=False)
    return out
